# revision 2
# baseline (speedup 1.0000x reference)
"""Two-layer GAT on 8 Trainium2 NeuronCores (Bass/Tile) — v2.

Strategy (edge/data parallel, dst-sharded, global node table):
  - Core c owns dst range [npc*c, npc*(c+1)) and processes exactly the edges
    pointing into it (plus self-loops).
  - Phase 0: each core computes ITS OWN npc rows of the layer-1 node table
    (x @ W1ext) then AllGathers to a Shared global table T1g [N, 384]
    (row = [feat(256) | alpha_src(4) | pad]).  Local D1 [npc, 128] holds
    alpha_dst for own dsts (cols 0:4).
  - Edge phase per dst window (128 dsts): dma_gather per-edge rows from T1g
    (A/B halves, int16 index limit), dma_gather alpha_dst rows from D1,
    batched DVE ops (stride-0 broadcast APs) for logits/one-hot/weighting,
    one PE matmul per 128-edge chunk accumulating [features | denom] in PSUM.
  - Layer-1 flush computes h2 rows + layer-2 table rows; T2own AllGathered to
    T2g [N, 256] (row = [feat(160) | alpha_src2(4) | pad]); layer 2 repeats
    the edge phase with the SAME index streams.

Numerics: tables/messages bf16, accumulation f32 PSUM, softmax without
max-subtraction (logits O(1), exact in f32).
"""
import math
import os
import sys
import types

sys.path.insert(0, "/opt/trn_rl_repo")

import numpy as np
import ml_dtypes

BF16 = ml_dtypes.bfloat16

FULL_CFG = dict(
    N=50000, E=800000, F_IN=128, HID=64, HEADS=4, NCLS=40, NEG=0.2,
    CORES=8, QUEUES=4,
)
WIN = 128
PADLOC = 200.0


def _wrap_idx(idx):
    """int16 gather-index layout: index j at [j%16, j//16], replicated to all
    eight 16-partition groups."""
    idx = np.asarray(idx, dtype=np.int16)
    n16 = max((len(idx) + 15) // 16, 1)
    arr = np.zeros((128, n16), dtype=np.int16)
    w = np.zeros(n16 * 16, dtype=np.int16)
    w[:len(idx)] = idx
    w = w.reshape(-1, 16).T
    for g in range(8):
        arr[g * 16:(g + 1) * 16, :] = w
    return arr


def build_schedules(src, dst, cfg):
    """Host-side edge schedule (shared by both layers): per-core gather index
    streams with a common compile-time chunk structure."""
    N, C = cfg["N"], cfg["CORES"]
    npc = N // C
    SPLIT = N // 2
    W = (npc + WIN - 1) // WIN

    cores = []
    for c in range(C):
        start = c * npc
        m = (dst >= start) & (dst < start + npc)
        cores.append((src[m], dst[m] - start))

    cnt = np.zeros((C, W, 2), dtype=np.int64)
    for c in range(C):
        g, dl = cores[c]
        np.add.at(cnt[c], (dl // WIN, (g >= SPLIT).astype(np.int64)), 1)
    CH = (np.max(cnt, axis=0) + 127) // 128          # [W, 2] chunks

    common = dict(W=W, npc=npc, CH=CH, SPLIT=SPLIT)

    per_core = []
    for c in range(C):
        g, dl_all = cores[c]
        srcA, srcB, dstl, dloc = [], [], [], []
        for w in range(W):
            wm = (dl_all // WIN) == w
            for h in (0, 1):
                sel = wm & ((g >= SPLIT) == bool(h))
                gi = g[sel] - h * SPLIT
                di = dl_all[sel]
                n_pad = int(CH[w, h]) * 128
                assert len(gi) <= n_pad
                gp = np.zeros(n_pad, np.int64); gp[:len(gi)] = gi
                dp = np.zeros(n_pad, np.int64); dp[:len(di)] = di
                lp = np.full(n_pad, PADLOC, np.float32)
                lp[:len(di)] = di - w * WIN
                (srcA if h == 0 else srcB).append(gp)
                dstl.append(dp)
                dloc.append(lp)
        cat = lambda xs: np.concatenate(xs) if xs else np.zeros(0, np.int64)
        dlc = np.concatenate(dloc)
        data = {
            "idxA": _wrap_idx(cat(srcA)),
            "idxB": _wrap_idx(cat(srcB)),
            "idxD": _wrap_idx(cat(dstl)),
            "dl": np.ascontiguousarray(
                dlc.reshape(-1, 128).T).astype(BF16),
        }
        per_core.append(data)
    return common, per_core


def fold_params(W1, a_src1, a_dst1, W2, a_src2, a_dst2, cfg):
    """W*_ext column layout: [features (head-major) | as-fold | ad-fold]."""
    H, HID, NCLS = cfg["HEADS"], cfg["HID"], cfg["NCLS"]
    f1 = [W1[:, h * HID:(h + 1) * HID] @ a_src1[h] for h in range(H)]
    f2 = [W1[:, h * HID:(h + 1) * HID] @ a_dst1[h] for h in range(H)]
    W1_ext = np.concatenate([W1] + [np.stack(f1, 1), np.stack(f2, 1)], axis=1)
    g1 = [W2[:, h * NCLS:(h + 1) * NCLS] @ a_src2[h] for h in range(H)]
    g2 = [W2[:, h * NCLS:(h + 1) * NCLS] @ a_dst2[h] for h in range(H)]
    W2_ext = np.concatenate([W2] + [np.stack(g1, 1), np.stack(g2, 1)], axis=1)
    return np.ascontiguousarray(W1_ext), np.ascontiguousarray(W2_ext)


# ---------------------------------------------------------------------------
# device program
# ---------------------------------------------------------------------------

def _ceil_mult(x, m):
    return (x + m - 1) // m * m


def build_program(cfg, common):
    import concourse.bacc as bacc
    import concourse.bass as bass
    import concourse.mybir as mybir
    import concourse.tile as tile
    from concourse.tile import ScopedClock

    # walrus in this toolchain rejects >1 sync-wait on the tail Drain; stage
    # waits on single-wait nops instead.
    def _drain_patch(self, tick_clock, wait_clock):
        nc_ = self.nc
        probe = nc_.sync.nop(nofuse=True)
        wait_clock.add_sem_waits(probe.ins,
                                 ScopedClock({None: tick_clock.global_clock}))
        si = probe.ins.sync_info
        if si is not None and si.on_wait is not None and len(si.on_wait) > 1:
            extra = list(si.on_wait[1:])
            si.on_wait = si.on_wait[:1]
            for wt in extra:
                n2 = nc_.sync.nop(nofuse=True)
                si2 = n2.ins.sync_info
                if si2 is None:
                    n2.ins.sync_info = mybir.SyncInfo(on_wait=[wt], on_update=[])
                else:
                    lst = si2.on_wait or []
                    lst.append(wt)
                    si2.on_wait = lst
        nc_.sync.drain()
        nc_.all_engine_barrier()
        popped = nc_._tile_sem_poison_stack.pop()
        assert popped is self._sem_poison
        nc_.clear_and_free_semaphores(list(self.sems.allocated().values()))
        nc_.all_engine_barrier()
    tile.TileContext._drain_and_barrier = _drain_patch

    N, C = cfg["N"], cfg["CORES"]
    H, HID, NCLS, F_IN = cfg["HEADS"], cfg["HID"], cfg["NCLS"], cfg["F_IN"]
    NEG = cfg["NEG"]
    npc, W, CH, SPLIT = common["npc"], common["W"], common["CH"], common["SPLIT"]
    F1, F2 = H * HID, H * NCLS                       # 256 / 160
    T1W = _ceil_mult(F1 + H, 128)                    # 384
    T2W = _ceil_mult(F2 + H, 128)                    # 256
    DW = 128
    NTD = (npc + 127) // 128
    NC_ = int(CH.sum())
    LA = int(CH[:, 0].sum()) * 128
    LB = int(CH[:, 1].sum()) * 128
    cAmax = int(CH[:, 0].max())
    cBmax = int(CH[:, 1].max())
    cTmax = int((CH[:, 0] + CH[:, 1]).max())
    bf16, f32, i16 = mybir.dt.bfloat16, mybir.dt.float32, mybir.dt.int16
    AF = mybir.ActivationFunctionType
    OP = mybir.AluOpType

    nc = bacc.Bacc("TRN2", target_bir_lowering=False, debug=False,
                   num_swdge_queues=cfg["QUEUES"])

    # ---- I/O ----
    xT = nc.dram_tensor("xT", [F_IN, npc], bf16, kind="ExternalInput")
    w1e = nc.dram_tensor("w1e", [F_IN, F1 + 2 * H], bf16, kind="ExternalInput")
    w2e = nc.dram_tensor("w2e", [HID, F2 + 2 * H], bf16, kind="ExternalInput")
    b1t = nc.dram_tensor("b1t", [128, HID], f32, kind="ExternalInput")
    b2t = nc.dram_tensor("b2t", [128, NCLS], f32, kind="ExternalInput")
    iota_d = nc.dram_tensor("iota", [128, 128], bf16, kind="ExternalInput")
    ident_d = nc.dram_tensor("ident", [128, 128], f32, kind="ExternalInput")
    idx_d = {}
    for nm, L in (("idxA", LA), ("idxB", LB), ("idxD", NC_ * 128)):
        idx_d[nm] = nc.dram_tensor(nm, [128, max(L // 16, 1)], i16,
                                   kind="ExternalInput")
    dl_d = nc.dram_tensor("dl", [128, NC_], bf16, kind="ExternalInput")
    out_d = nc.dram_tensor("out", [W * 128, NCLS], f32, kind="ExternalOutput")

    from contextlib import ExitStack
    ctx = ExitStack()
    with tile.TileContext(nc) as tc, ctx:
        cpool = ctx.enter_context(tc.tile_pool(name="const", bufs=1))
        gpool = ctx.enter_context(tc.tile_pool(name="gather", bufs=2))
        spool = ctx.enter_context(tc.tile_pool(name="small", bufs=2))
        wpool = ctx.enter_context(tc.tile_pool(name="work", bufs=2))
        pspool = ctx.enter_context(tc.tile_pool(name="ps", bufs=2, space="PSUM"))
        ps2pool = ctx.enter_context(tc.tile_pool(name="ps2", bufs=2, space="PSUM"))
        dpool = ctx.enter_context(tc.tile_pool(name="dram", bufs=1, space="DRAM"))

        T1own = dpool.tile([npc, T1W], bf16)
        D1 = dpool.tile([npc, DW], bf16)
        T2own = dpool.tile([npc, T2W], bf16)
        D2 = dpool.tile([npc, DW], bf16)
        T1g = dpool.tile([npc * C, T1W], bf16, addr_space="Shared")
        T2g = dpool.tile([npc * C, T2W], bf16, addr_space="Shared")

        def load_const(dram, shape, dtype, tag):
            t = cpool.tile(shape, dtype, tag=tag)
            nc.sync.dma_start(out=t[:], in_=dram[:])
            return t

        w1_t = load_const(w1e, [F_IN, F1 + 2 * H], bf16, "c_w1")
        w2_t = load_const(w2e, [HID, F2 + 2 * H], bf16, "c_w2")
        b1_t = load_const(b1t, [128, HID], f32, "c_b1")
        b2_t = load_const(b2t, [128, NCLS], f32, "c_b2")
        iota_t = load_const(iota_d, [128, 128], bf16, "c_iota")
        ident_t = load_const(ident_d, [128, 128], f32, "c_id")
        idx_t = {nm: load_const(d, list(d.shape), i16, "c_" + nm)
                 for nm, d in idx_d.items()}
        dl_t = load_const(dl_d, [128, NC_], bf16, "c_dl")

        # ---------------- phase 0: own table rows (sharded) ----------------
        # xT is the per-core pre-sliced [F_IN, npc] block of x.T.
        zt = cpool.tile([128, 128], bf16, tag="c_zero")
        nc.vector.memset(zt[:], 0.0)
        for t in range(NTD):
            n0 = t * 128
            nn_ = min(128, npc - n0)
            xt = wpool.tile([F_IN, 128], bf16, tag="xt")
            nc.sync.dma_start(out=xt[:, :nn_], in_=xT[:, n0:n0 + nn_])
            ps = pspool.tile([128, F1 + 2 * H], f32, tag="psA")
            nc.tensor.matmul(out=ps[:nn_, :], lhsT=xt[:, :nn_], rhs=w1_t[:],
                             start=True, stop=True)
            t1 = wpool.tile([128, F1 + H], bf16, tag="t1")
            half = (F1 + H) // 2
            nc.vector.tensor_copy(t1[:nn_, :half], ps[:nn_, :half])
            nc.scalar.copy(t1[:nn_, half:], ps[:nn_, half:F1 + H])
            nc.sync.dma_start(out=T1own[n0:n0 + nn_, 0:F1 + H], in_=t1[:nn_, :])
            nc.sync.dma_start(out=T1own[n0:n0 + nn_, F1 + H:T1W],
                              in_=zt[:nn_, :T1W - F1 - H])
            d1 = wpool.tile([128, H], bf16, tag="d1")
            nc.scalar.copy(d1[:nn_, :], ps[:nn_, F1 + H:F1 + 2 * H])
            nc.sync.dma_start(out=D1[n0:n0 + nn_, 0:H], in_=d1[:nn_, :])
            nc.sync.dma_start(out=D1[n0:n0 + nn_, H:DW], in_=zt[:nn_, :DW - H])

        nc.gpsimd.collective_compute(
            "AllGather", mybir.AluOpType.bypass,
            replica_groups=[list(range(C))],
            ins=[T1own[:, :]], outs=[T1g[:, :]])

        # ---------------- edge phase (shared for both layers) --------------
        def edge_layer(layer, Tsrc, Dsrc, FW, TW, flush):
            pos = {"A": 0, "B": 0, "ci": 0}
            qn = [0]
            rows = Tsrc.shape[0]
            for w in range(W):
                cA, cB = int(CH[w, 0]), int(CH[w, 1])
                cT = cA + cB
                if cT == 0:
                    flush(w, None)
                    continue
                tiles = []
                for half, cnt in (("A", cA), ("B", cB)):
                    if cnt == 0:
                        tiles.append(None)
                        continue
                    gt = gpool.tile([128, cnt, TW], bf16, tag=f"g{layer}{half}")
                    p = pos[half]
                    it = idx_t["idxA" if half == "A" else "idxB"]
                    base = (Tsrc[0:SPLIT, :] if half == "A"
                            else Tsrc[SPLIT:rows, :])
                    nc.gpsimd.dma_gather(
                        gt[:], base,
                        it[:, p:p + 8 * cnt], cnt * 128, cnt * 128, TW,
                        single_packet=False, queue_num=qn[0] % cfg["QUEUES"])
                    qn[0] += 1
                    pos[half] += cnt * 8
                    tiles.append(gt)
                ci0 = pos["ci"]
                dt = gpool.tile([128, cT, DW], bf16, tag=f"d{layer}")
                nc.gpsimd.dma_gather(
                    dt[:], Dsrc[:, :], idx_t["idxD"][:, ci0 * 8:(ci0 + cT) * 8],
                    cT * 128, cT * 128, DW,
                    single_packet=False, queue_num=qn[0] % cfg["QUEUES"])
                qn[0] += 1

                # ---- batched logits: wv = exp(lrelu(as+ad)) ----
                wv = spool.tile([128, cT, H], f32, tag=f"wv{layer}")
                off = 0
                for gt, cnt in ((tiles[0], cA), (tiles[1], cB)):
                    if cnt == 0:
                        continue
                    nc.vector.tensor_tensor(
                        out=wv[:, off:off + cnt, :],
                        in0=gt[:, :, FW:FW + H],
                        in1=dt[:, off:off + cnt, 0:H],
                        op=OP.add)
                    off += cnt
                lrt = spool.tile([128, cT, H], f32, tag=f"lrt{layer}")
                nc.vector.tensor_scalar(out=lrt[:], in0=wv[:],
                                        scalar1=NEG, scalar2=None, op0=OP.mult)
                nc.vector.tensor_tensor(out=wv[:], in0=wv[:], in1=lrt[:],
                                        op=OP.max)
                nc.scalar.activation(wv[:], wv[:], AF.Exp)
                wvb = spool.tile([128, cT, H], bf16, tag=f"wvb{layer}")
                nc.scalar.copy(wvb[:], wv[:])

                # ---- gw = g * w  (+ wvb tail for the denominator) ----
                CWH = FW // H
                gw = wpool.tile([128, cT, FW + H], bf16, tag=f"gw{layer}")
                off = 0
                for gt, cnt in ((tiles[0], cA), (tiles[1], cB)):
                    if cnt == 0:
                        continue
                    for h in range(H):
                        nc.vector.tensor_tensor(
                            out=gw[:, off:off + cnt, h * CWH:(h + 1) * CWH],
                            in0=gt[:, :, h * CWH:(h + 1) * CWH],
                            in1=wvb[:, off:off + cnt, h:h + 1]
                                .broadcast_to([128, cnt, CWH]),
                            op=OP.mult)
                    off += cnt
                nc.scalar.copy(gw[:, :, FW:FW + H], wvb[:])

                # ---- batched one-hot S ----
                S = wpool.tile([128, cT, 128], bf16, tag=f"S{layer}")
                nc.vector.tensor_tensor(
                    out=S[:],
                    in0=iota_t[:].unsqueeze(1).broadcast_to([128, cT, 128]),
                    in1=dl_t[:, ci0:ci0 + cT].unsqueeze(2)
                        .broadcast_to([128, cT, 128]),
                    op=OP.is_equal)

                # ---- aggregation matmuls ----
                psw = pspool.tile([128, FW + H], f32, tag="psA")
                for ci in range(cT):
                    nc.tensor.matmul(out=psw[:], lhsT=S[:, ci, :],
                                     rhs=gw[:, ci, :],
                                     start=(ci == 0), stop=(ci == cT - 1))
                pos["ci"] += cT
                flush(w, psw)

        # ---------------- layer 1 flush: h2-table rows ---------------------
        def flush1(w, psw):
            n0 = w * 128
            nrow = max(min(128, npc - n0), 0)
            if nrow == 0 or psw is None:
                return
            den = spool.tile([128, H], f32, tag="den1")
            nc.vector.tensor_scalar(out=den[:], in0=psw[:, F1:F1 + H],
                                    scalar1=1e-30, scalar2=None, op0=OP.max)
            rec = spool.tile([128, H], f32, tag="rec1")
            nc.vector.reciprocal(rec[:], den[:])
            acc = wpool.tile([128, HID], f32, tag="acc1")
            tmp = wpool.tile([128, HID], f32, tag="tmp1")
            for h in range(H):
                dstt = acc if h == 0 else tmp
                nc.vector.tensor_scalar(
                    out=dstt[:], in0=psw[:, h * HID:(h + 1) * HID],
                    scalar1=rec[:, h:h + 1], scalar2=1.0 / H,
                    op0=OP.mult, op1=OP.mult)
                if h > 0:
                    nc.vector.tensor_tensor(out=acc[:], in0=acc[:], in1=tmp[:],
                                            op=OP.add)
            nc.vector.tensor_tensor(out=acc[:], in0=acc[:], in1=b1_t[:, :HID],
                                    op=OP.add)
            r1 = wpool.tile([128, HID], f32, tag="r1")
            nc.scalar.activation(r1[:], acc[:], AF.Relu)
            psT = ps2pool.tile([HID, 128], f32, tag="psB")
            nc.tensor.transpose(out=psT[:], in_=r1[:], identity=ident_t[:])
            l1T = wpool.tile([HID, 128], bf16, tag="l1T")
            nc.scalar.copy(l1T[:], psT[:])
            ps2 = ps2pool.tile([128, F2 + 2 * H], f32, tag="psB")
            nc.tensor.matmul(out=ps2[:], lhsT=l1T[:], rhs=w2_t[:],
                             start=True, stop=True)
            t2 = wpool.tile([128, F2 + H], bf16, tag="t2")
            nc.vector.tensor_copy(t2[:nrow, :], ps2[:nrow, 0:F2 + H])
            d2 = wpool.tile([128, H], bf16, tag="d2")
            nc.scalar.copy(d2[:nrow, :], ps2[:nrow, F2 + H:F2 + 2 * H])
            nc.sync.dma_start(out=T2own[n0:n0 + nrow, 0:F2 + H], in_=t2[:nrow, :])
            nc.sync.dma_start(out=T2own[n0:n0 + nrow, F2 + H:T2W],
                              in_=zt[:nrow, :T2W - F2 - H])
            nc.sync.dma_start(out=D2[n0:n0 + nrow, 0:H], in_=d2[:nrow, :])
            nc.sync.dma_start(out=D2[n0:n0 + nrow, H:DW], in_=zt[:nrow, :DW - H])

        # ---------------- layer 2 flush: final output ----------------------
        def flush2(w, psw):
            n0 = w * 128
            nrow = max(min(128, npc - n0), 0)
            if nrow == 0:
                return
            o2 = wpool.tile([128, NCLS], f32, tag="o2")
            if psw is None:
                nc.vector.memset(o2[:], 0.0)
            else:
                den = spool.tile([128, H], f32, tag="den2")
                nc.vector.tensor_scalar(out=den[:], in0=psw[:, F2:F2 + H],
                                        scalar1=1e-30, scalar2=None, op0=OP.max)
                rec = spool.tile([128, H], f32, tag="rec2")
                nc.vector.reciprocal(rec[:], den[:])
                tmp = wpool.tile([128, NCLS], f32, tag="tmp2")
                for h in range(H):
                    dstt = o2 if h == 0 else tmp
                    nc.vector.tensor_scalar(
                        out=dstt[:], in0=psw[:, h * NCLS:(h + 1) * NCLS],
                        scalar1=rec[:, h:h + 1], scalar2=1.0 / H,
                        op0=OP.mult, op1=OP.mult)
                    if h > 0:
                        nc.vector.tensor_tensor(out=o2[:], in0=o2[:],
                                                in1=tmp[:], op=OP.add)
                nc.vector.tensor_tensor(out=o2[:], in0=o2[:], in1=b2_t[:, :NCLS],
                                        op=OP.add)
            nc.sync.dma_start(out=out_d[n0:n0 + nrow, :], in_=o2[:nrow, :])

        edge_layer(1, T1g, D1, F1, T1W, flush1)

        nc.gpsimd.collective_compute(
            "AllGather", mybir.AluOpType.bypass,
            replica_groups=[list(range(C))],
            ins=[T2own[:, :]], outs=[T2g[:, :]])

        edge_layer(2, T2g, D2, F2, T2W, flush2)

    nc.compile()
    return nc


# ---------------------------------------------------------------------------
# host driver
# ---------------------------------------------------------------------------

def _install_ntff_hook():
    try:
        from trn_agent_boot.trn_boot import _ntff_profile_via_ctypes
        hook = _ntff_profile_via_ctypes("/opt/axon/libaxon_pjrt.so")
        m = types.ModuleType("antenv.axon_hooks")
        m.get_axon_ntff_profile_hook = lambda: hook
        m.set_axon_ntff_profile_hook = lambda h: None
        sys.modules["antenv.axon_hooks"] = m
    except Exception:
        pass


def make_inputs(x, edge_index, W1, a_src1, a_dst1, b1, W2, a_src2, a_dst2, b2,
                cfg):
    N, C = cfg["N"], cfg["CORES"]
    npc = N // C
    loops = np.arange(N, dtype=np.int64)
    src = np.concatenate([np.asarray(edge_index[0]), loops])
    dst = np.concatenate([np.asarray(edge_index[1]), loops])
    common, per_core = build_schedules(src, dst, cfg)
    W1e, W2e = fold_params(np.asarray(W1, np.float32), np.asarray(a_src1),
                           np.asarray(a_dst1), np.asarray(W2),
                           np.asarray(a_src2), np.asarray(a_dst2), cfg)
    iota = np.broadcast_to(np.arange(128, dtype=np.float32), (128, 128))
    ident = np.eye(128, dtype=np.float32)
    b1t = np.broadcast_to(np.asarray(b1, np.float32), (128, cfg["HID"]))
    b2t = np.broadcast_to(np.asarray(b2, np.float32), (128, cfg["NCLS"]))
    x = np.asarray(x, np.float32)
    xT_all = np.ascontiguousarray(x.T).astype(BF16)       # [F_IN, N]
    in_maps = []
    for c in range(C):
        start = c * npc
        m = dict(per_core[c])
        m["xT"] = np.ascontiguousarray(xT_all[:, start:start + npc])
        m["w1e"] = W1e.astype(BF16)
        m["w2e"] = W2e.astype(BF16)
        m["b1t"] = np.ascontiguousarray(b1t)
        m["b2t"] = np.ascontiguousarray(b2t)
        m["iota"] = np.ascontiguousarray(iota).astype(BF16)
        m["ident"] = ident
        in_maps.append(m)
    return common, in_maps


def kernel(x, edge_index, W1, a_src1, a_dst1, b1, W2, a_src2, a_dst2, b2,
           cfg=None, trace=False, sim=False):
    cfg = cfg or FULL_CFG
    _install_ntff_hook()

    common, in_maps = make_inputs(x, edge_index, W1, a_src1, a_dst1, b1,
                                  W2, a_src2, a_dst2, b2, cfg)
    nc = build_program(cfg, common)
    C, npc = cfg["CORES"], cfg["N"] // cfg["CORES"]

    if sim:
        import concourse.bass_interp as bass_interp
        s = bass_interp.MultiCoreSim(nc, C)
        for c in range(C):
            for k, v in in_maps[c].items():
                s.cores[c].tensor(k)[:] = v
        s.simulate()
        outs = [np.array(s.cores[c].tensor("out")) for c in range(C)]
        kernel.last_exec_ns = None
    else:
        from concourse.bass_utils import run_bass_kernel_spmd
        res = run_bass_kernel_spmd(nc, in_maps, list(range(C)), trace=trace)
        outs = [res.results[c]["out"] for c in range(C)]
        kernel.last_exec_ns = res.exec_time_ns
    return np.concatenate([o[:npc] for o in outs], axis=0)


# revision 3
# speedup vs baseline: 1.0024x; 1.0024x over previous
"""Two-layer GAT on 8 Trainium2 NeuronCores (Bass/Tile) — v3.

Deltas vs v2:
  - alpha_dst gather ELIMINATED: the one-hot S (built anyway) is transposed on
    the PE (identity trick) to St [dst-local, edge-slot]; per-edge alpha_dst is
    then St^T @ alpha_dst_window, a tiny matmul against the bulk-loaded
    [128, H] window slice of the local D table.  This removes 1/3 of the
    dma_gather rows (the Q7 descriptor-generation bottleneck: ~5.6 ns/row).
  - AllGathers are split into 7 chunks each, fired as their producing
    phase-0 tiles / layer-1 flushes complete, so the transfers overlap
    compute.  The shared tables use a chunk-major layout
    [chunk][core][rows_in_chunk] with a host-side row map.
"""
import math
import os
import sys
import types

sys.path.insert(0, "/opt/trn_rl_repo")

import numpy as np
import ml_dtypes

BF16 = ml_dtypes.bfloat16

FULL_CFG = dict(
    N=50000, E=800000, F_IN=128, HID=64, HEADS=4, NCLS=40, NEG=0.2,
    CORES=8, QUEUES=4,
)
WIN = 128
PADLOC = 200.0
TPC = 49         # tiles per AllGather chunk (49 = single collective; Shared tiles allow only one writer)


def _wrap_idx(idx):
    idx = np.asarray(idx, dtype=np.int16)
    n16 = max((len(idx) + 15) // 16, 1)
    arr = np.zeros((128, n16), dtype=np.int16)
    w = np.zeros(n16 * 16, dtype=np.int16)
    w[:len(idx)] = idx
    w = w.reshape(-1, 16).T
    for g in range(8):
        arr[g * 16:(g + 1) * 16, :] = w
    return arr


def _chunk_geometry(npc):
    """Chunk sizes (rows per core) for the 7-way AllGather split."""
    W = (npc + WIN - 1) // WIN
    nck = (W + TPC - 1) // TPC
    sizes = []
    for k in range(nck):
        lo = k * TPC * WIN
        hi = min((k + 1) * TPC * WIN, npc)
        sizes.append(hi - lo)
    cum = np.concatenate([[0], np.cumsum([s * 8 for s in sizes])])
    return nck, sizes, cum


def build_schedules(src, dst, cfg):
    N, C = cfg["N"], cfg["CORES"]
    npc = N // C
    SPLIT = N // 2
    W = (npc + WIN - 1) // WIN
    nck, sizes, cum = _chunk_geometry(npc)

    # row map: global node id -> row in the chunk-major shared table
    g_all = np.arange(N, dtype=np.int64)
    cc = g_all // npc
    jj = g_all % npc
    KK = np.minimum(jj // (TPC * WIN), nck - 1)
    sz = np.array(sizes, dtype=np.int64)
    rowmap = cum[KK] + cc * sz[KK] + (jj - KK * TPC * WIN)

    cores = []
    for c in range(C):
        start = c * npc
        m = (dst >= start) & (dst < start + npc)
        cores.append((rowmap[src[m]], dst[m] - start))

    cnt = np.zeros((C, W, 2), dtype=np.int64)
    for c in range(C):
        g, dl = cores[c]
        np.add.at(cnt[c], (dl // WIN, (g >= SPLIT).astype(np.int64)), 1)
    CH = (np.max(cnt, axis=0) + 127) // 128

    common = dict(W=W, npc=npc, CH=CH, SPLIT=SPLIT, nck=nck, sizes=sizes,
                  cum=cum)

    per_core = []
    for c in range(C):
        g, dl_all = cores[c]
        srcA, srcB, dloc = [], [], []
        for w in range(W):
            wm = (dl_all // WIN) == w
            for h in (0, 1):
                sel = wm & ((g >= SPLIT) == bool(h))
                gi = g[sel] - h * SPLIT
                di = dl_all[sel]
                n_pad = int(CH[w, h]) * 128
                assert len(gi) <= n_pad
                gp = np.zeros(n_pad, np.int64); gp[:len(gi)] = gi
                lp = np.full(n_pad, PADLOC, np.float32)
                lp[:len(di)] = di - w * WIN
                (srcA if h == 0 else srcB).append(gp)
                dloc.append(lp)
        cat = lambda xs: np.concatenate(xs) if xs else np.zeros(0, np.int64)
        dlc = np.concatenate(dloc)
        per_core.append({
            "idxA": _wrap_idx(cat(srcA)),
            "idxB": _wrap_idx(cat(srcB)),
            "dl": np.ascontiguousarray(dlc.reshape(-1, 128).T).astype(BF16),
        })
    return common, per_core


def fold_params(W1, a_src1, a_dst1, W2, a_src2, a_dst2, cfg):
    H, HID, NCLS = cfg["HEADS"], cfg["HID"], cfg["NCLS"]
    f1 = [W1[:, h * HID:(h + 1) * HID] @ a_src1[h] for h in range(H)]
    f2 = [W1[:, h * HID:(h + 1) * HID] @ a_dst1[h] for h in range(H)]
    W1_ext = np.concatenate([W1] + [np.stack(f1, 1), np.stack(f2, 1)], axis=1)
    g1 = [W2[:, h * NCLS:(h + 1) * NCLS] @ a_src2[h] for h in range(H)]
    g2 = [W2[:, h * NCLS:(h + 1) * NCLS] @ a_dst2[h] for h in range(H)]
    W2_ext = np.concatenate([W2] + [np.stack(g1, 1), np.stack(g2, 1)], axis=1)
    return np.ascontiguousarray(W1_ext), np.ascontiguousarray(W2_ext)


def _ceil_mult(x, m):
    return (x + m - 1) // m * m


def build_program(cfg, common):
    import concourse.bacc as bacc
    import concourse.bass as bass
    import concourse.mybir as mybir
    import concourse.tile as tile
    from concourse.tile import ScopedClock

    def _drain_patch(self, tick_clock, wait_clock):
        nc_ = self.nc
        probe = nc_.sync.nop(nofuse=True)
        wait_clock.add_sem_waits(probe.ins,
                                 ScopedClock({None: tick_clock.global_clock}))
        si = probe.ins.sync_info
        if si is not None and si.on_wait is not None and len(si.on_wait) > 1:
            extra = list(si.on_wait[1:])
            si.on_wait = si.on_wait[:1]
            for wt in extra:
                n2 = nc_.sync.nop(nofuse=True)
                si2 = n2.ins.sync_info
                if si2 is None:
                    n2.ins.sync_info = mybir.SyncInfo(on_wait=[wt], on_update=[])
                else:
                    lst = si2.on_wait or []
                    lst.append(wt)
                    si2.on_wait = lst
        nc_.sync.drain()
        nc_.all_engine_barrier()
        popped = nc_._tile_sem_poison_stack.pop()
        assert popped is self._sem_poison
        nc_.clear_and_free_semaphores(list(self.sems.allocated().values()))
        nc_.all_engine_barrier()
    tile.TileContext._drain_and_barrier = _drain_patch

    N, C = cfg["N"], cfg["CORES"]
    H, HID, NCLS, F_IN = cfg["HEADS"], cfg["HID"], cfg["NCLS"], cfg["F_IN"]
    NEG = cfg["NEG"]
    npc, W, CH, SPLIT = common["npc"], common["W"], common["CH"], common["SPLIT"]
    nck, sizes, cum = common["nck"], common["sizes"], common["cum"]
    F1, F2 = H * HID, H * NCLS
    T1W = _ceil_mult(F1 + H, 128)                    # 384
    T2W = _ceil_mult(F2 + H, 128)                    # 256
    NTD = (npc + 127) // 128
    NC_ = int(CH.sum())
    LA = int(CH[:, 0].sum()) * 128
    LB = int(CH[:, 1].sum()) * 128
    bf16, f32, i16 = mybir.dt.bfloat16, mybir.dt.float32, mybir.dt.int16
    AF = mybir.ActivationFunctionType
    OP = mybir.AluOpType

    nc = bacc.Bacc("TRN2", target_bir_lowering=False, debug=False,
                   num_swdge_queues=cfg["QUEUES"])

    xT = nc.dram_tensor("xT", [F_IN, npc], bf16, kind="ExternalInput")
    w1e = nc.dram_tensor("w1e", [F_IN, F1 + 2 * H], bf16, kind="ExternalInput")
    w2e = nc.dram_tensor("w2e", [HID, F2 + 2 * H], bf16, kind="ExternalInput")
    b1t = nc.dram_tensor("b1t", [128, HID], f32, kind="ExternalInput")
    b2t = nc.dram_tensor("b2t", [128, NCLS], f32, kind="ExternalInput")
    iota_d = nc.dram_tensor("iota", [128, 128], bf16, kind="ExternalInput")
    ident_d = nc.dram_tensor("ident", [128, 128], f32, kind="ExternalInput")
    identb_d = nc.dram_tensor("identb", [128, 128], bf16, kind="ExternalInput")
    idx_d = {}
    for nm, L in (("idxA", LA), ("idxB", LB)):
        idx_d[nm] = nc.dram_tensor(nm, [128, max(L // 16, 1)], i16,
                                   kind="ExternalInput")
    dl_d = nc.dram_tensor("dl", [128, NC_], bf16, kind="ExternalInput")
    out_d = nc.dram_tensor("out", [W * 128, NCLS], f32, kind="ExternalOutput")

    from contextlib import ExitStack
    ctx = ExitStack()
    with tile.TileContext(nc) as tc, ctx:
        cpool = ctx.enter_context(tc.tile_pool(name="const", bufs=1))
        gpool = ctx.enter_context(tc.tile_pool(name="gather", bufs=2))
        spool = ctx.enter_context(tc.tile_pool(name="small", bufs=2))
        wpool = ctx.enter_context(tc.tile_pool(name="work", bufs=2))
        pspool = ctx.enter_context(tc.tile_pool(name="ps", bufs=2, space="PSUM"))
        pstpool = ctx.enter_context(tc.tile_pool(name="pst", bufs=2, space="PSUM"))
        ps2pool = ctx.enter_context(tc.tile_pool(name="ps2", bufs=1, space="PSUM"))
        dpool = ctx.enter_context(tc.tile_pool(name="dram", bufs=1, space="DRAM"))

        T1own = dpool.tile([npc, T1W], bf16)
        D1 = dpool.tile([npc, H], bf16)
        T2own = dpool.tile([npc, T2W], bf16)
        D2 = dpool.tile([npc, H], bf16)
        T1g = dpool.tile([N, T1W], bf16, addr_space="Shared")
        T2g = dpool.tile([N, T2W], bf16, addr_space="Shared")

        def load_const(dram, shape, dtype, tag):
            t = cpool.tile(shape, dtype, tag=tag)
            nc.sync.dma_start(out=t[:], in_=dram[:])
            return t

        w1_t = load_const(w1e, [F_IN, F1 + 2 * H], bf16, "c_w1")
        w2_t = load_const(w2e, [HID, F2 + 2 * H], bf16, "c_w2")
        b1_t = load_const(b1t, [128, HID], f32, "c_b1")
        b2_t = load_const(b2t, [128, NCLS], f32, "c_b2")
        iota_t = load_const(iota_d, [128, 128], bf16, "c_iota")
        ident_t = load_const(ident_d, [128, 128], f32, "c_id")
        identb_t = load_const(identb_d, [128, 128], bf16, "c_idb")
        idx_t = {nm: load_const(d, list(d.shape), i16, "c_" + nm)
                 for nm, d in idx_d.items()}
        dl_t = load_const(dl_d, [128, NC_], bf16, "c_dl")

        def ag_chunk(k, Town, Tg, TWd):
            lo = k * TPC * WIN
            sz = sizes[k]
            nc.gpsimd.collective_compute(
                "AllGather", mybir.AluOpType.bypass,
                replica_groups=[list(range(C))],
                ins=[Town[lo:lo + sz, :]],
                outs=[Tg[int(cum[k]):int(cum[k]) + C * sz, :]])

        # ---------------- phase 0: own table rows (sharded) ----------------
        zt = cpool.tile([128, 128], bf16, tag="c_zero")
        nc.vector.memset(zt[:], 0.0)
        for t in range(NTD):
            n0 = t * 128
            nn_ = min(128, npc - n0)
            xt = wpool.tile([F_IN, 128], bf16, tag="xt")
            nc.sync.dma_start(out=xt[:, :nn_], in_=xT[:, n0:n0 + nn_])
            ps = pspool.tile([128, F1 + 2 * H], f32, tag="psA")
            nc.tensor.matmul(out=ps[:nn_, :], lhsT=xt[:, :nn_], rhs=w1_t[:],
                             start=True, stop=True)
            t1 = wpool.tile([128, F1 + H], bf16, tag="t1")
            half = (F1 + H) // 2
            nc.vector.tensor_copy(t1[:nn_, :half], ps[:nn_, :half])
            nc.scalar.copy(t1[:nn_, half:], ps[:nn_, half:F1 + H])
            nc.sync.dma_start(out=T1own[n0:n0 + nn_, 0:F1 + H], in_=t1[:nn_, :])
            nc.sync.dma_start(out=T1own[n0:n0 + nn_, F1 + H:T1W],
                              in_=zt[:nn_, :T1W - F1 - H])
            d1 = wpool.tile([128, H], bf16, tag="d1")
            nc.scalar.copy(d1[:nn_, :], ps[:nn_, F1 + H:F1 + 2 * H])
            nc.sync.dma_start(out=D1[n0:n0 + nn_, :], in_=d1[:nn_, :])
            if t % TPC == TPC - 1 or t == NTD - 1:
                ag_chunk(t // TPC, T1own, T1g, T1W)

        # ---------------- edge phase (shared for both layers) --------------
        def edge_layer(layer, Tsrc, Dsrc, FW, TW, flush, after_window=None):
            pos = {"A": 0, "B": 0, "ci": 0}
            qn = [0]
            rows = Tsrc.shape[0]
            for w in range(W):
                cA, cB = int(CH[w, 0]), int(CH[w, 1])
                cT = cA + cB
                nrow = max(min(128, npc - w * 128), 0)
                if cT == 0:
                    flush(w, None)
                    if after_window is not None:
                        after_window(w)
                    continue
                tiles = []
                for half, cnt in (("A", cA), ("B", cB)):
                    if cnt == 0:
                        tiles.append(None)
                        continue
                    gt = gpool.tile([128, cnt, TW], bf16, tag=f"g{layer}{half}")
                    p = pos[half]
                    it = idx_t["idxA" if half == "A" else "idxB"]
                    base = (Tsrc[0:SPLIT, :] if half == "A"
                            else Tsrc[SPLIT:rows, :])
                    nc.gpsimd.dma_gather(
                        gt[:], base,
                        it[:, p:p + 8 * cnt], cnt * 128, cnt * 128, TW,
                        single_packet=False, queue_num=qn[0] % cfg["QUEUES"])
                    qn[0] += 1
                    pos[half] += cnt * 8
                    tiles.append(gt)
                ci0 = pos["ci"]

                # ---- batched one-hot S [edge-slot partition, (chunk, j)] ----
                S = wpool.tile([128, cT, 128], bf16, tag=f"S{layer}")
                nc.vector.tensor_tensor(
                    out=S[:],
                    in0=iota_t[:].unsqueeze(1).broadcast_to([128, cT, 128]),
                    in1=dl_t[:, ci0:ci0 + cT].unsqueeze(2)
                        .broadcast_to([128, cT, 128]),
                    op=OP.is_equal)

                # ---- St = S^T per chunk (PE transpose via identity) --------
                St = wpool.tile([128, cT, 128], bf16, tag=f"St{layer}")
                for k0 in range(0, cT, 4):
                    g_ = min(4, cT - k0)
                    psSt = pstpool.tile([128, 4, 128], bf16, tag="psSt")
                    for i in range(g_):
                        nc.tensor.transpose(out=psSt[:, i, :],
                                            in_=S[:, k0 + i, :],
                                            identity=identb_t[:])
                    if (k0 // 4) % 2 == 0:
                        nc.scalar.copy(St[:, k0:k0 + g_, :], psSt[:, 0:g_, :])
                    else:
                        nc.vector.tensor_copy(St[:, k0:k0 + g_, :],
                                              psSt[:, 0:g_, :])

                # ---- per-edge alpha_dst via St @ alpha_dst_window ----------
                adw = spool.tile([128, H], bf16, tag=f"adw{layer}")
                if nrow < 128:
                    nc.vector.memset(adw[:], 0.0)
                nc.sync.dma_start(out=adw[:nrow, :],
                                  in_=Dsrc[w * 128:w * 128 + nrow, :])
                pad = pstpool.tile([128, cT * H], f32, tag="psad")
                for ci in range(cT):
                    nc.tensor.matmul(out=pad[:, ci * H:(ci + 1) * H],
                                     lhsT=St[:, ci, :], rhs=adw[:],
                                     start=True, stop=True)

                # ---- batched logits: wv = exp(lrelu(as+ad)) ----------------
                wv = spool.tile([128, cT, H], f32, tag=f"wv{layer}")
                off = 0
                for gt, cnt in ((tiles[0], cA), (tiles[1], cB)):
                    if cnt == 0:
                        continue
                    nc.vector.tensor_tensor(
                        out=wv[:, off:off + cnt, :],
                        in0=gt[:, :, FW:FW + H],
                        in1=pad[:].rearrange("p (a b) -> p a b", b=H)
                            [:, off:off + cnt, :],
                        op=OP.add)
                    off += cnt
                lrt = spool.tile([128, cT, H], f32, tag=f"lrt{layer}")
                nc.vector.tensor_scalar(out=lrt[:], in0=wv[:],
                                        scalar1=NEG, scalar2=None, op0=OP.mult)
                nc.vector.tensor_tensor(out=wv[:], in0=wv[:], in1=lrt[:],
                                        op=OP.max)
                nc.scalar.activation(wv[:], wv[:], AF.Exp)
                wvb = spool.tile([128, cT, H], bf16, tag=f"wvb{layer}")
                nc.scalar.copy(wvb[:], wv[:])

                # ---- gw = g * w  (+ wvb tail for the denominator) ----------
                CWH = FW // H
                gw = wpool.tile([128, cT, FW + H], bf16, tag=f"gw{layer}")
                off = 0
                for gt, cnt in ((tiles[0], cA), (tiles[1], cB)):
                    if cnt == 0:
                        continue
                    for h in range(H):
                        nc.vector.tensor_tensor(
                            out=gw[:, off:off + cnt, h * CWH:(h + 1) * CWH],
                            in0=gt[:, :, h * CWH:(h + 1) * CWH],
                            in1=wvb[:, off:off + cnt, h:h + 1]
                                .broadcast_to([128, cnt, CWH]),
                            op=OP.mult)
                    off += cnt
                nc.scalar.copy(gw[:, :, FW:FW + H], wvb[:])

                # ---- aggregation matmuls -----------------------------------
                psw = pspool.tile([128, FW + H], f32, tag="psA")
                for ci in range(cT):
                    nc.tensor.matmul(out=psw[:], lhsT=S[:, ci, :],
                                     rhs=gw[:, ci, :],
                                     start=(ci == 0), stop=(ci == cT - 1))
                pos["ci"] += cT
                flush(w, psw)
                if after_window is not None:
                    after_window(w)

        # ---------------- layer 1 flush: h2-table rows ---------------------
        def flush1(w, psw):
            n0 = w * 128
            nrow = max(min(128, npc - n0), 0)
            if nrow == 0 or psw is None:
                return
            den = spool.tile([128, H], f32, tag="den1")
            nc.vector.tensor_scalar(out=den[:], in0=psw[:, F1:F1 + H],
                                    scalar1=1e-30, scalar2=None, op0=OP.max)
            rec = spool.tile([128, H], f32, tag="rec1")
            nc.vector.reciprocal(rec[:], den[:])
            acc = wpool.tile([128, HID], f32, tag="acc1")
            tmp = wpool.tile([128, HID], f32, tag="tmp1")
            for h in range(H):
                dstt = acc if h == 0 else tmp
                nc.vector.tensor_scalar(
                    out=dstt[:], in0=psw[:, h * HID:(h + 1) * HID],
                    scalar1=rec[:, h:h + 1], scalar2=1.0 / H,
                    op0=OP.mult, op1=OP.mult)
                if h > 0:
                    nc.vector.tensor_tensor(out=acc[:], in0=acc[:], in1=tmp[:],
                                            op=OP.add)
            nc.vector.tensor_tensor(out=acc[:], in0=acc[:], in1=b1_t[:, :HID],
                                    op=OP.add)
            r1 = wpool.tile([128, HID], f32, tag="r1")
            nc.scalar.activation(r1[:], acc[:], AF.Relu)
            psT = ps2pool.tile([HID, 128], f32, tag="psB")
            nc.tensor.transpose(out=psT[:], in_=r1[:], identity=ident_t[:])
            l1T = wpool.tile([HID, 128], bf16, tag="l1T")
            nc.scalar.copy(l1T[:], psT[:])
            ps2 = ps2pool.tile([128, F2 + 2 * H], f32, tag="psC")
            nc.tensor.matmul(out=ps2[:], lhsT=l1T[:], rhs=w2_t[:],
                             start=True, stop=True)
            t2 = wpool.tile([128, F2 + H], bf16, tag="t2")
            nc.vector.tensor_copy(t2[:nrow, :], ps2[:nrow, 0:F2 + H])
            d2 = wpool.tile([128, H], bf16, tag="d2")
            nc.scalar.copy(d2[:nrow, :], ps2[:nrow, F2 + H:F2 + 2 * H])
            nc.sync.dma_start(out=T2own[n0:n0 + nrow, 0:F2 + H], in_=t2[:nrow, :])
            nc.sync.dma_start(out=T2own[n0:n0 + nrow, F2 + H:T2W],
                              in_=zt[:nrow, :T2W - F2 - H])
            nc.sync.dma_start(out=D2[n0:n0 + nrow, :], in_=d2[:nrow, :])

        def after1(w):
            if w % TPC == TPC - 1 or w == W - 1:
                ag_chunk(w // TPC, T2own, T2g, T2W)

        # ---------------- layer 2 flush: final output ----------------------
        def flush2(w, psw):
            n0 = w * 128
            nrow = max(min(128, npc - n0), 0)
            if nrow == 0:
                return
            o2 = wpool.tile([128, NCLS], f32, tag="o2")
            if psw is None:
                nc.vector.memset(o2[:], 0.0)
            else:
                den = spool.tile([128, H], f32, tag="den2")
                nc.vector.tensor_scalar(out=den[:], in0=psw[:, F2:F2 + H],
                                        scalar1=1e-30, scalar2=None, op0=OP.max)
                rec = spool.tile([128, H], f32, tag="rec2")
                nc.vector.reciprocal(rec[:], den[:])
                tmp = wpool.tile([128, NCLS], f32, tag="tmp2")
                for h in range(H):
                    dstt = o2 if h == 0 else tmp
                    nc.vector.tensor_scalar(
                        out=dstt[:], in0=psw[:, h * NCLS:(h + 1) * NCLS],
                        scalar1=rec[:, h:h + 1], scalar2=1.0 / H,
                        op0=OP.mult, op1=OP.mult)
                    if h > 0:
                        nc.vector.tensor_tensor(out=o2[:], in0=o2[:],
                                                in1=tmp[:], op=OP.add)
                nc.vector.tensor_tensor(out=o2[:], in0=o2[:], in1=b2_t[:, :NCLS],
                                        op=OP.add)
            nc.sync.dma_start(out=out_d[n0:n0 + nrow, :], in_=o2[:nrow, :])

        edge_layer(1, T1g, D1, F1, T1W, flush1, after_window=after1)
        edge_layer(2, T2g, D2, F2, T2W, flush2)

    nc.compile()
    return nc


# ---------------------------------------------------------------------------
# host driver
# ---------------------------------------------------------------------------

def _install_ntff_hook():
    try:
        from trn_agent_boot.trn_boot import _ntff_profile_via_ctypes
        hook = _ntff_profile_via_ctypes("/opt/axon/libaxon_pjrt.so")
        m = types.ModuleType("antenv.axon_hooks")
        m.get_axon_ntff_profile_hook = lambda: hook
        m.set_axon_ntff_profile_hook = lambda h: None
        sys.modules["antenv.axon_hooks"] = m
    except Exception:
        pass


def make_inputs(x, edge_index, W1, a_src1, a_dst1, b1, W2, a_src2, a_dst2, b2,
                cfg):
    N, C = cfg["N"], cfg["CORES"]
    npc = N // C
    loops = np.arange(N, dtype=np.int64)
    src = np.concatenate([np.asarray(edge_index[0]), loops])
    dst = np.concatenate([np.asarray(edge_index[1]), loops])
    common, per_core = build_schedules(src, dst, cfg)
    W1e, W2e = fold_params(np.asarray(W1, np.float32), np.asarray(a_src1),
                           np.asarray(a_dst1), np.asarray(W2),
                           np.asarray(a_src2), np.asarray(a_dst2), cfg)
    iota = np.broadcast_to(np.arange(128, dtype=np.float32), (128, 128))
    ident = np.eye(128, dtype=np.float32)
    b1t = np.broadcast_to(np.asarray(b1, np.float32), (128, cfg["HID"]))
    b2t = np.broadcast_to(np.asarray(b2, np.float32), (128, cfg["NCLS"]))
    x = np.asarray(x, np.float32)
    xT_all = np.ascontiguousarray(x.T).astype(BF16)
    in_maps = []
    for c in range(C):
        start = c * npc
        m = dict(per_core[c])
        m["xT"] = np.ascontiguousarray(xT_all[:, start:start + npc])
        m["w1e"] = W1e.astype(BF16)
        m["w2e"] = W2e.astype(BF16)
        m["b1t"] = np.ascontiguousarray(b1t)
        m["b2t"] = np.ascontiguousarray(b2t)
        m["iota"] = np.ascontiguousarray(iota).astype(BF16)
        m["ident"] = ident
        m["identb"] = ident.astype(BF16)
        in_maps.append(m)
    return common, in_maps


def kernel(x, edge_index, W1, a_src1, a_dst1, b1, W2, a_src2, a_dst2, b2,
           cfg=None, trace=False, sim=False):
    cfg = cfg or FULL_CFG
    _install_ntff_hook()

    common, in_maps = make_inputs(x, edge_index, W1, a_src1, a_dst1, b1,
                                  W2, a_src2, a_dst2, b2, cfg)
    nc = build_program(cfg, common)
    C, npc = cfg["CORES"], cfg["N"] // cfg["CORES"]

    if sim:
        import concourse.bass_interp as bass_interp
        s = bass_interp.MultiCoreSim(nc, C)
        for c in range(C):
            for k, v in in_maps[c].items():
                s.cores[c].tensor(k)[:] = v
        s.simulate()
        outs = [np.array(s.cores[c].tensor("out")) for c in range(C)]
        kernel.last_exec_ns = None
    else:
        from concourse.bass_utils import run_bass_kernel_spmd
        res = run_bass_kernel_spmd(nc, in_maps, list(range(C)), trace=trace)
        outs = [res.results[c]["out"] for c in range(C)]
        kernel.last_exec_ns = res.exec_time_ns
    return np.concatenate([o[:npc] for o in outs], axis=0)


# revision 4
# speedup vs baseline: 1.2324x; 1.2294x over previous
"""Two-layer GAT on 8 Trainium2 NeuronCores (Bass/Tile) — v3.

Deltas vs v2:
  - alpha_dst gather ELIMINATED: the one-hot S (built anyway) is transposed on
    the PE (identity trick) to St [dst-local, edge-slot]; per-edge alpha_dst is
    then St^T @ alpha_dst_window, a tiny matmul against the bulk-loaded
    [128, H] window slice of the local D table.  This removes 1/3 of the
    dma_gather rows (the Q7 descriptor-generation bottleneck: ~5.6 ns/row).
  - AllGathers are split into 7 chunks each, fired as their producing
    phase-0 tiles / layer-1 flushes complete, so the transfers overlap
    compute.  The shared tables use a chunk-major layout
    [chunk][core][rows_in_chunk] with a host-side row map.
"""
import math
import os
import sys
import types

sys.path.insert(0, "/opt/trn_rl_repo")

import numpy as np
import ml_dtypes

BF16 = ml_dtypes.bfloat16

FULL_CFG = dict(
    N=50000, E=800000, F_IN=128, HID=64, HEADS=4, NCLS=40, NEG=0.2,
    CORES=8, QUEUES=4,
)
WIN = 128
PADLOC = 200.0
TPC = 25         # tiles per AllGather chunk (2 chunks -> 2 Shared tiles,
                 # one collective each; chunk 0 overlaps the back half)


def _wrap_idx(idx):
    idx = np.asarray(idx, dtype=np.int16)
    n16 = max((len(idx) + 15) // 16, 1)
    arr = np.zeros((128, n16), dtype=np.int16)
    w = np.zeros(n16 * 16, dtype=np.int16)
    w[:len(idx)] = idx
    w = w.reshape(-1, 16).T
    for g in range(8):
        arr[g * 16:(g + 1) * 16, :] = w
    return arr


def _chunk_geometry(npc):
    """Chunk sizes (rows per core) for the 7-way AllGather split."""
    W = (npc + WIN - 1) // WIN
    nck = (W + TPC - 1) // TPC
    sizes = []
    for k in range(nck):
        lo = k * TPC * WIN
        hi = min((k + 1) * TPC * WIN, npc)
        sizes.append(hi - lo)
    cum = np.concatenate([[0], np.cumsum([s * 8 for s in sizes])])
    return nck, sizes, cum


def build_schedules(src, dst, cfg):
    N, C = cfg["N"], cfg["CORES"]
    npc = N // C
    W = (npc + WIN - 1) // WIN
    nck, sizes, cum = _chunk_geometry(npc)
    SPLIT = int(cum[1]) if nck == 2 else N // 2

    # row map: global node id -> row in the chunk-major shared table
    g_all = np.arange(N, dtype=np.int64)
    cc = g_all // npc
    jj = g_all % npc
    KK = np.minimum(jj // (TPC * WIN), nck - 1)
    sz = np.array(sizes, dtype=np.int64)
    rowmap = cum[KK] + cc * sz[KK] + (jj - KK * TPC * WIN)

    cores = []
    for c in range(C):
        start = c * npc
        m = (dst >= start) & (dst < start + npc)
        cores.append((rowmap[src[m]], dst[m] - start))

    cnt = np.zeros((C, W, 2), dtype=np.int64)
    for c in range(C):
        g, dl = cores[c]
        np.add.at(cnt[c], (dl // WIN, (g >= SPLIT).astype(np.int64)), 1)
    CH = (np.max(cnt, axis=0) + 127) // 128

    common = dict(W=W, npc=npc, CH=CH, SPLIT=SPLIT, nck=nck, sizes=sizes,
                  cum=cum)

    per_core = []
    for c in range(C):
        g, dl_all = cores[c]
        srcA, srcB, dloc = [], [], []
        for w in range(W):
            wm = (dl_all // WIN) == w
            for h in (0, 1):
                sel = wm & ((g >= SPLIT) == bool(h))
                gi = g[sel] - h * SPLIT
                di = dl_all[sel]
                n_pad = int(CH[w, h]) * 128
                assert len(gi) <= n_pad
                gp = np.zeros(n_pad, np.int64); gp[:len(gi)] = gi
                lp = np.full(n_pad, PADLOC, np.float32)
                lp[:len(di)] = di - w * WIN
                (srcA if h == 0 else srcB).append(gp)
                dloc.append(lp)
        cat = lambda xs: np.concatenate(xs) if xs else np.zeros(0, np.int64)
        dlc = np.concatenate(dloc)
        per_core.append({
            "idxA": _wrap_idx(cat(srcA)),
            "idxB": _wrap_idx(cat(srcB)),
            "dl": np.ascontiguousarray(dlc.reshape(-1, 128).T).astype(BF16),
        })
    return common, per_core


def fold_params(W1, a_src1, a_dst1, W2, a_src2, a_dst2, cfg):
    H, HID, NCLS = cfg["HEADS"], cfg["HID"], cfg["NCLS"]
    f1 = [W1[:, h * HID:(h + 1) * HID] @ a_src1[h] for h in range(H)]
    f2 = [W1[:, h * HID:(h + 1) * HID] @ a_dst1[h] for h in range(H)]
    W1_ext = np.concatenate([W1] + [np.stack(f1, 1), np.stack(f2, 1)], axis=1)
    g1 = [W2[:, h * NCLS:(h + 1) * NCLS] @ a_src2[h] for h in range(H)]
    g2 = [W2[:, h * NCLS:(h + 1) * NCLS] @ a_dst2[h] for h in range(H)]
    W2_ext = np.concatenate([W2] + [np.stack(g1, 1), np.stack(g2, 1)], axis=1)
    return np.ascontiguousarray(W1_ext), np.ascontiguousarray(W2_ext)


def _ceil_mult(x, m):
    return (x + m - 1) // m * m


def build_program(cfg, common):
    import concourse.bacc as bacc
    import concourse.bass as bass
    import concourse.mybir as mybir
    import concourse.tile as tile
    from concourse.tile import ScopedClock

    def _drain_patch(self, tick_clock, wait_clock):
        nc_ = self.nc
        probe = nc_.sync.nop(nofuse=True)
        wait_clock.add_sem_waits(probe.ins,
                                 ScopedClock({None: tick_clock.global_clock}))
        si = probe.ins.sync_info
        if si is not None and si.on_wait is not None and len(si.on_wait) > 1:
            extra = list(si.on_wait[1:])
            si.on_wait = si.on_wait[:1]
            for wt in extra:
                n2 = nc_.sync.nop(nofuse=True)
                si2 = n2.ins.sync_info
                if si2 is None:
                    n2.ins.sync_info = mybir.SyncInfo(on_wait=[wt], on_update=[])
                else:
                    lst = si2.on_wait or []
                    lst.append(wt)
                    si2.on_wait = lst
        nc_.sync.drain()
        nc_.all_engine_barrier()
        popped = nc_._tile_sem_poison_stack.pop()
        assert popped is self._sem_poison
        nc_.clear_and_free_semaphores(list(self.sems.allocated().values()))
        nc_.all_engine_barrier()
    tile.TileContext._drain_and_barrier = _drain_patch

    N, C = cfg["N"], cfg["CORES"]
    H, HID, NCLS, F_IN = cfg["HEADS"], cfg["HID"], cfg["NCLS"], cfg["F_IN"]
    NEG = cfg["NEG"]
    npc, W, CH, SPLIT = common["npc"], common["W"], common["CH"], common["SPLIT"]
    nck, sizes, cum = common["nck"], common["sizes"], common["cum"]
    assert nck in (1, 2)
    F1, F2 = H * HID, H * NCLS
    T1W = _ceil_mult(F1 + H, 128)                    # 384
    T2W = _ceil_mult(F2 + H, 128)                    # 256
    NTD = (npc + 127) // 128
    NC_ = int(CH.sum())
    LA = int(CH[:, 0].sum()) * 128
    LB = int(CH[:, 1].sum()) * 128
    bf16, f32, i16 = mybir.dt.bfloat16, mybir.dt.float32, mybir.dt.int16
    AF = mybir.ActivationFunctionType
    OP = mybir.AluOpType

    nc = bacc.Bacc("TRN2", target_bir_lowering=False, debug=False,
                   num_swdge_queues=cfg["QUEUES"])

    xT = nc.dram_tensor("xT", [F_IN, npc], bf16, kind="ExternalInput")
    w1e = nc.dram_tensor("w1e", [F_IN, F1 + 2 * H], bf16, kind="ExternalInput")
    w2e = nc.dram_tensor("w2e", [HID, F2 + 2 * H], bf16, kind="ExternalInput")
    b1t = nc.dram_tensor("b1t", [128, HID], f32, kind="ExternalInput")
    b2t = nc.dram_tensor("b2t", [128, NCLS], f32, kind="ExternalInput")
    iota_d = nc.dram_tensor("iota", [128, 128], bf16, kind="ExternalInput")
    ident_d = nc.dram_tensor("ident", [128, 128], f32, kind="ExternalInput")
    identb_d = nc.dram_tensor("identb", [128, 128], bf16, kind="ExternalInput")
    idx_d = {}
    for nm, L in (("idxA", LA), ("idxB", LB)):
        idx_d[nm] = nc.dram_tensor(nm, [128, max(L // 16, 1)], i16,
                                   kind="ExternalInput")
    dl_d = nc.dram_tensor("dl", [128, NC_], bf16, kind="ExternalInput")
    out_d = nc.dram_tensor("out", [W * 128, NCLS], f32, kind="ExternalOutput")

    from contextlib import ExitStack
    ctx = ExitStack()
    with tile.TileContext(nc) as tc, ctx:
        cpool = ctx.enter_context(tc.tile_pool(name="const", bufs=1))
        gpool = ctx.enter_context(tc.tile_pool(name="gather", bufs=2))
        spool = ctx.enter_context(tc.tile_pool(name="small", bufs=2))
        wpool = ctx.enter_context(tc.tile_pool(name="work", bufs=2))
        pspool = ctx.enter_context(tc.tile_pool(name="ps", bufs=2, space="PSUM"))
        pstpool = ctx.enter_context(tc.tile_pool(name="pst", bufs=2, space="PSUM"))
        ps2pool = ctx.enter_context(tc.tile_pool(name="ps2", bufs=1, space="PSUM"))
        dpool = ctx.enter_context(tc.tile_pool(name="dram", bufs=1, space="DRAM"))

        T1own = dpool.tile([npc, T1W], bf16)
        D1 = dpool.tile([npc, H], bf16)
        T2own = dpool.tile([npc, T2W], bf16)
        D2 = dpool.tile([npc, H], bf16)
        if nck == 2:
            T1gA = dpool.tile([SPLIT, T1W], bf16, addr_space="Shared")
            T1gB = dpool.tile([N - SPLIT, T1W], bf16, addr_space="Shared")
            T2gA = dpool.tile([SPLIT, T2W], bf16, addr_space="Shared")
            T2gB = dpool.tile([N - SPLIT, T2W], bf16, addr_space="Shared")
        else:
            T1gA = T1gB = dpool.tile([N, T1W], bf16, addr_space="Shared")
            T2gA = T2gB = dpool.tile([N, T2W], bf16, addr_space="Shared")

        def load_const(dram, shape, dtype, tag):
            t = cpool.tile(shape, dtype, tag=tag)
            nc.sync.dma_start(out=t[:], in_=dram[:])
            return t

        w1_t = load_const(w1e, [F_IN, F1 + 2 * H], bf16, "c_w1")
        w2_t = load_const(w2e, [HID, F2 + 2 * H], bf16, "c_w2")
        b1_t = load_const(b1t, [128, HID], f32, "c_b1")
        b2_t = load_const(b2t, [128, NCLS], f32, "c_b2")
        iota_t = load_const(iota_d, [128, 128], bf16, "c_iota")
        ident_t = load_const(ident_d, [128, 128], f32, "c_id")
        identb_t = load_const(identb_d, [128, 128], bf16, "c_idb")
        idx_t = {nm: load_const(d, list(d.shape), i16, "c_" + nm)
                 for nm, d in idx_d.items()}
        dl_t = load_const(dl_d, [128, NC_], bf16, "c_dl")

        def ag_chunk(k, Town, TgA, TgB):
            lo = k * TPC * WIN
            sz = sizes[k]
            if nck == 2:
                Tg = TgA if k == 0 else TgB
                nc.gpsimd.collective_compute(
                    "AllGather", mybir.AluOpType.bypass,
                    replica_groups=[list(range(C))],
                    ins=[Town[lo:lo + sz, :]], outs=[Tg[0:C * sz, :]])
            else:
                nc.gpsimd.collective_compute(
                    "AllGather", mybir.AluOpType.bypass,
                    replica_groups=[list(range(C))],
                    ins=[Town[lo:lo + sz, :]],
                    outs=[TgA[int(cum[k]):int(cum[k]) + C * sz, :]])

        # ---------------- phase 0: own table rows (sharded) ----------------
        zt = cpool.tile([128, 128], bf16, tag="c_zero")
        nc.vector.memset(zt[:], 0.0)
        for t in range(NTD):
            n0 = t * 128
            nn_ = min(128, npc - n0)
            xt = wpool.tile([F_IN, 128], bf16, tag="xt")
            nc.sync.dma_start(out=xt[:, :nn_], in_=xT[:, n0:n0 + nn_])
            ps = pspool.tile([128, F1 + 2 * H], f32, tag="psA")
            nc.tensor.matmul(out=ps[:nn_, :], lhsT=xt[:, :nn_], rhs=w1_t[:],
                             start=True, stop=True)
            t1 = wpool.tile([128, F1 + H], bf16, tag="t1")
            half = (F1 + H) // 2
            nc.vector.tensor_copy(t1[:nn_, :half], ps[:nn_, :half])
            nc.scalar.copy(t1[:nn_, half:], ps[:nn_, half:F1 + H])
            nc.sync.dma_start(out=T1own[n0:n0 + nn_, 0:F1 + H], in_=t1[:nn_, :])
            nc.sync.dma_start(out=T1own[n0:n0 + nn_, F1 + H:T1W],
                              in_=zt[:nn_, :T1W - F1 - H])
            d1 = wpool.tile([128, H], bf16, tag="d1")
            nc.scalar.copy(d1[:nn_, :], ps[:nn_, F1 + H:F1 + 2 * H])
            nc.sync.dma_start(out=D1[n0:n0 + nn_, :], in_=d1[:nn_, :])
            if t % TPC == TPC - 1 or t == NTD - 1:
                ag_chunk(t // TPC, T1own, T1gA, T1gB)

        # ---------------- edge phase (shared for both layers) --------------
        def edge_layer(layer, TsrcA, TsrcB, Dsrc, FW, TW, flush,
                       after_window=None):
            pos = {"A": 0, "B": 0, "ci": 0}
            qn = [0]
            for w in range(W):
                cA, cB = int(CH[w, 0]), int(CH[w, 1])
                cT = cA + cB
                nrow = max(min(128, npc - w * 128), 0)
                if cT == 0:
                    flush(w, None)
                    if after_window is not None:
                        after_window(w)
                    continue
                tiles = []
                for half, cnt in (("A", cA), ("B", cB)):
                    if cnt == 0:
                        tiles.append(None)
                        continue
                    gt = gpool.tile([128, cnt, TW], bf16, tag=f"g{layer}{half}")
                    p = pos[half]
                    it = idx_t["idxA" if half == "A" else "idxB"]
                    if nck == 2:
                        base = TsrcA[:, :] if half == "A" else TsrcB[:, :]
                    else:
                        base = (TsrcA[0:SPLIT, :] if half == "A"
                                else TsrcB[SPLIT:TsrcB.shape[0], :])
                    nc.gpsimd.dma_gather(
                        gt[:], base,
                        it[:, p:p + 8 * cnt], cnt * 128, cnt * 128, TW,
                        single_packet=False, queue_num=qn[0] % cfg["QUEUES"])
                    qn[0] += 1
                    pos[half] += cnt * 8
                    tiles.append(gt)
                ci0 = pos["ci"]

                # ---- batched one-hot S [edge-slot partition, (chunk, j)] ----
                S = wpool.tile([128, cT, 128], bf16, tag=f"S{layer}")
                nc.vector.tensor_tensor(
                    out=S[:],
                    in0=iota_t[:].unsqueeze(1).broadcast_to([128, cT, 128]),
                    in1=dl_t[:, ci0:ci0 + cT].unsqueeze(2)
                        .broadcast_to([128, cT, 128]),
                    op=OP.is_equal)

                # ---- St = S^T per chunk (PE transpose via identity) --------
                St = wpool.tile([128, cT, 128], bf16, tag=f"St{layer}")
                for k0 in range(0, cT, 4):
                    g_ = min(4, cT - k0)
                    psSt = pstpool.tile([128, 4, 128], bf16, tag="psSt")
                    for i in range(g_):
                        nc.tensor.transpose(out=psSt[:, i, :],
                                            in_=S[:, k0 + i, :],
                                            identity=identb_t[:])
                    nc.scalar.copy(St[:, k0:k0 + g_, :], psSt[:, 0:g_, :])

                # ---- per-edge alpha_dst via St @ alpha_dst_window ----------
                adw = spool.tile([128, H], bf16, tag=f"adw{layer}")
                if nrow < 128:
                    nc.vector.memset(adw[:], 0.0)
                nc.sync.dma_start(out=adw[:nrow, :],
                                  in_=Dsrc[w * 128:w * 128 + nrow, :])
                pad = pstpool.tile([128, cT * H], f32, tag="psad")
                for ci in range(cT):
                    nc.tensor.matmul(out=pad[:, ci * H:(ci + 1) * H],
                                     lhsT=St[:, ci, :], rhs=adw[:],
                                     start=True, stop=True)

                # ---- batched logits: wv = exp(lrelu(as+ad)) ----------------
                wv = spool.tile([128, cT, H], f32, tag=f"wv{layer}")
                off = 0
                for gt, cnt in ((tiles[0], cA), (tiles[1], cB)):
                    if cnt == 0:
                        continue
                    nc.vector.tensor_tensor(
                        out=wv[:, off:off + cnt, :],
                        in0=gt[:, :, FW:FW + H],
                        in1=pad[:].rearrange("p (a b) -> p a b", b=H)
                            [:, off:off + cnt, :],
                        op=OP.add)
                    off += cnt
                lrt = spool.tile([128, cT, H], f32, tag=f"lrt{layer}")
                nc.vector.tensor_scalar(out=lrt[:], in0=wv[:],
                                        scalar1=NEG, scalar2=None, op0=OP.mult)
                nc.vector.tensor_tensor(out=wv[:], in0=wv[:], in1=lrt[:],
                                        op=OP.max)
                nc.scalar.activation(wv[:], wv[:], AF.Exp)
                wvb = spool.tile([128, cT, H], bf16, tag=f"wvb{layer}")
                nc.scalar.copy(wvb[:], wv[:])

                # ---- gw = g * w  (+ wvb tail for the denominator) ----------
                CWH = FW // H
                gw = wpool.tile([128, cT, FW + H], bf16, tag=f"gw{layer}")
                off = 0
                for gt, cnt in ((tiles[0], cA), (tiles[1], cB)):
                    if cnt == 0:
                        continue
                    for h in range(H):
                        nc.vector.tensor_tensor(
                            out=gw[:, off:off + cnt, h * CWH:(h + 1) * CWH],
                            in0=gt[:, :, h * CWH:(h + 1) * CWH],
                            in1=wvb[:, off:off + cnt, h:h + 1]
                                .broadcast_to([128, cnt, CWH]),
                            op=OP.mult)
                    off += cnt
                nc.scalar.copy(gw[:, :, FW:FW + H], wvb[:])

                # ---- aggregation matmuls -----------------------------------
                psw = pspool.tile([128, FW + H], f32, tag="psA")
                for ci in range(cT):
                    nc.tensor.matmul(out=psw[:], lhsT=S[:, ci, :],
                                     rhs=gw[:, ci, :],
                                     start=(ci == 0), stop=(ci == cT - 1))
                pos["ci"] += cT
                flush(w, psw)
                if after_window is not None:
                    after_window(w)

        # ---------------- layer 1 flush: h2-table rows ---------------------
        def flush1(w, psw):
            n0 = w * 128
            nrow = max(min(128, npc - n0), 0)
            if nrow == 0 or psw is None:
                return
            den = spool.tile([128, H], f32, tag="den1")
            nc.vector.tensor_scalar(out=den[:], in0=psw[:, F1:F1 + H],
                                    scalar1=1e-30, scalar2=None, op0=OP.max)
            rec = spool.tile([128, H], f32, tag="rec1")
            nc.vector.reciprocal(rec[:], den[:])
            acc = wpool.tile([128, HID], f32, tag="acc1")
            tmp = wpool.tile([128, HID], f32, tag="tmp1")
            for h in range(H):
                dstt = acc if h == 0 else tmp
                nc.vector.tensor_scalar(
                    out=dstt[:], in0=psw[:, h * HID:(h + 1) * HID],
                    scalar1=rec[:, h:h + 1], scalar2=1.0 / H,
                    op0=OP.mult, op1=OP.mult)
                if h > 0:
                    nc.vector.tensor_tensor(out=acc[:], in0=acc[:], in1=tmp[:],
                                            op=OP.add)
            nc.vector.tensor_tensor(out=acc[:], in0=acc[:], in1=b1_t[:, :HID],
                                    op=OP.add)
            r1 = wpool.tile([128, HID], f32, tag="r1")
            nc.scalar.activation(r1[:], acc[:], AF.Relu)
            psT = ps2pool.tile([HID, 128], f32, tag="psB")
            nc.tensor.transpose(out=psT[:], in_=r1[:], identity=ident_t[:])
            l1T = wpool.tile([HID, 128], bf16, tag="l1T")
            nc.scalar.copy(l1T[:], psT[:])
            ps2 = ps2pool.tile([128, F2 + 2 * H], f32, tag="psC")
            nc.tensor.matmul(out=ps2[:], lhsT=l1T[:], rhs=w2_t[:],
                             start=True, stop=True)
            t2 = wpool.tile([128, F2 + H], bf16, tag="t2")
            nc.vector.tensor_copy(t2[:nrow, :], ps2[:nrow, 0:F2 + H])
            d2 = wpool.tile([128, H], bf16, tag="d2")
            nc.scalar.copy(d2[:nrow, :], ps2[:nrow, F2 + H:F2 + 2 * H])
            nc.sync.dma_start(out=T2own[n0:n0 + nrow, 0:F2 + H], in_=t2[:nrow, :])
            nc.sync.dma_start(out=T2own[n0:n0 + nrow, F2 + H:T2W],
                              in_=zt[:nrow, :T2W - F2 - H])
            nc.sync.dma_start(out=D2[n0:n0 + nrow, :], in_=d2[:nrow, :])

        def after1(w):
            if w % TPC == TPC - 1 or w == W - 1:
                ag_chunk(w // TPC, T2own, T2gA, T2gB)

        # ---------------- layer 2 flush: final output ----------------------
        def flush2(w, psw):
            n0 = w * 128
            nrow = max(min(128, npc - n0), 0)
            if nrow == 0:
                return
            o2 = wpool.tile([128, NCLS], f32, tag="o2")
            if psw is None:
                nc.vector.memset(o2[:], 0.0)
            else:
                den = spool.tile([128, H], f32, tag="den2")
                nc.vector.tensor_scalar(out=den[:], in0=psw[:, F2:F2 + H],
                                        scalar1=1e-30, scalar2=None, op0=OP.max)
                rec = spool.tile([128, H], f32, tag="rec2")
                nc.vector.reciprocal(rec[:], den[:])
                tmp = wpool.tile([128, NCLS], f32, tag="tmp2")
                for h in range(H):
                    dstt = o2 if h == 0 else tmp
                    nc.vector.tensor_scalar(
                        out=dstt[:], in0=psw[:, h * NCLS:(h + 1) * NCLS],
                        scalar1=rec[:, h:h + 1], scalar2=1.0 / H,
                        op0=OP.mult, op1=OP.mult)
                    if h > 0:
                        nc.vector.tensor_tensor(out=o2[:], in0=o2[:],
                                                in1=tmp[:], op=OP.add)
                nc.vector.tensor_tensor(out=o2[:], in0=o2[:], in1=b2_t[:, :NCLS],
                                        op=OP.add)
            nc.sync.dma_start(out=out_d[n0:n0 + nrow, :], in_=o2[:nrow, :])

        edge_layer(1, T1gA, T1gB, D1, F1, T1W, flush1, after_window=after1)
        edge_layer(2, T2gA, T2gB, D2, F2, T2W, flush2)

    nc.compile()
    return nc


# ---------------------------------------------------------------------------
# host driver
# ---------------------------------------------------------------------------

def _install_ntff_hook():
    try:
        from trn_agent_boot.trn_boot import _ntff_profile_via_ctypes
        hook = _ntff_profile_via_ctypes("/opt/axon/libaxon_pjrt.so")
        m = types.ModuleType("antenv.axon_hooks")
        m.get_axon_ntff_profile_hook = lambda: hook
        m.set_axon_ntff_profile_hook = lambda h: None
        sys.modules["antenv.axon_hooks"] = m
    except Exception:
        pass


def make_inputs(x, edge_index, W1, a_src1, a_dst1, b1, W2, a_src2, a_dst2, b2,
                cfg):
    N, C = cfg["N"], cfg["CORES"]
    npc = N // C
    loops = np.arange(N, dtype=np.int64)
    src = np.concatenate([np.asarray(edge_index[0]), loops])
    dst = np.concatenate([np.asarray(edge_index[1]), loops])
    common, per_core = build_schedules(src, dst, cfg)
    W1e, W2e = fold_params(np.asarray(W1, np.float32), np.asarray(a_src1),
                           np.asarray(a_dst1), np.asarray(W2),
                           np.asarray(a_src2), np.asarray(a_dst2), cfg)
    iota = np.broadcast_to(np.arange(128, dtype=np.float32), (128, 128))
    ident = np.eye(128, dtype=np.float32)
    b1t = np.broadcast_to(np.asarray(b1, np.float32), (128, cfg["HID"]))
    b2t = np.broadcast_to(np.asarray(b2, np.float32), (128, cfg["NCLS"]))
    x = np.asarray(x, np.float32)
    xT_all = np.ascontiguousarray(x.T).astype(BF16)
    in_maps = []
    for c in range(C):
        start = c * npc
        m = dict(per_core[c])
        m["xT"] = np.ascontiguousarray(xT_all[:, start:start + npc])
        m["w1e"] = W1e.astype(BF16)
        m["w2e"] = W2e.astype(BF16)
        m["b1t"] = np.ascontiguousarray(b1t)
        m["b2t"] = np.ascontiguousarray(b2t)
        m["iota"] = np.ascontiguousarray(iota).astype(BF16)
        m["ident"] = ident
        m["identb"] = ident.astype(BF16)
        in_maps.append(m)
    return common, in_maps


def kernel(x, edge_index, W1, a_src1, a_dst1, b1, W2, a_src2, a_dst2, b2,
           cfg=None, trace=False, sim=False):
    cfg = cfg or FULL_CFG
    _install_ntff_hook()

    common, in_maps = make_inputs(x, edge_index, W1, a_src1, a_dst1, b1,
                                  W2, a_src2, a_dst2, b2, cfg)
    nc = build_program(cfg, common)
    C, npc = cfg["CORES"], cfg["N"] // cfg["CORES"]

    if sim:
        import concourse.bass_interp as bass_interp
        s = bass_interp.MultiCoreSim(nc, C)
        for c in range(C):
            for k, v in in_maps[c].items():
                s.cores[c].tensor(k)[:] = v
        s.simulate()
        outs = [np.array(s.cores[c].tensor("out")) for c in range(C)]
        kernel.last_exec_ns = None
    else:
        from concourse.bass_utils import run_bass_kernel_spmd
        res = run_bass_kernel_spmd(nc, in_maps, list(range(C)), trace=trace)
        outs = [res.results[c]["out"] for c in range(C)]
        kernel.last_exec_ns = res.exec_time_ns
    return np.concatenate([o[:npc] for o in outs], axis=0)
